# revision 7
# baseline (speedup 1.0000x reference)
"""QSP KAN forward on 8 Trainium2 NeuronCores (Bass, data-parallel).

Math: with 2d X-rotations (d=27 -> 54 W factors), <0|U|0> contains only
even harmonics of theta, so

    qsp(theta) = H(cos 2*theta),   H = degree-27 polynomial.

H's 28 Chebyshev coefficients follow from the 55 phases by interpolating
the 2x2 recurrence at 28 nodes (O(55^2) host work). H is factored into
13 real quadratics + 1 linear term (conjugate root pairs), each quadratic
rewritten as (sigma*v + beta)^2 + c so the Scalar engine's Square
activation evaluates it in one op. Per-factor scales are balanced on a
grid so every fp16 intermediate stays in [~1e-4, ~1]; a host-side fp16
simulation of the exact device arithmetic gates the fp16 path, falling
back to fp32 tiles if the draw is pathological.

Device per core (65536 elements as one [128, 512] tile):
  DVE : range-reduce 2*theta+pi/2 into [-pi, pi] (magic-number rint),
        then a 13-step fused (sq_i + c_i) * y chain (scalar_tensor_tensor),
        final multiplies with the linear factor and alphas.
  ACT : one Sin + 13 Square + 1 Copy, all in the single trig_and_small
        table set (one table load).
  DMA : x in, alphas in (cast to f16), out.  No collectives needed.
"""

import numpy as np
from numpy.polynomial import chebyshev as _cheb

import concourse.bass as bass
import concourse.mybir as mybir
from concourse.bass_utils import run_bass_kernel_spmd

QSP_DEPTH = 27
N_PHIS = 2 * QSP_DEPTH + 1  # 55
B = 524288
N_CORES = 8
P, F = 128, 512  # per-core tile; P*F == B/N_CORES

_PI = float(np.pi)
_MAGIC = float(1.5 * 2**23)  # fp32 round-to-nearest-int magic constant
dt = mybir.dt
AF = mybir.ActivationFunctionType
AL = mybir.AluOpType


def _qsp_f64(theta, phis):
    """Reference QSP expectation, float64 (first row of the 2x2 chain)."""
    c = np.cos(theta)
    s = np.sin(theta)
    r0r = np.ones_like(theta)
    r0i = np.zeros_like(theta)
    r1r = np.zeros_like(theta)
    r1i = np.zeros_like(theta)
    for phi in phis[1:]:
        cp, sp = np.cos(phi), np.sin(phi)
        ar = r0r * c - r1i * s
        ai = r0i * c + r1r * s
        br = r1r * c - r0i * s
        bi = r1i * c + r0r * s
        r0r = ar * cp - ai * sp
        r0i = ar * sp + ai * cp
        r1r = br * cp + bi * sp
        r1i = bi * cp - br * sp
    return r0r * np.cos(phis[0]) - r0i * np.sin(phis[0])


def _build_factors(phis):
    """Factor qsp(theta) = H(v), v = cos 2 theta, into balanced real factors.

    Returns (quads, lin, const) where
      quads: list of (sigma, beta, c) with factor_i(v) = (sigma*v+beta)^2 + c
      lin:   (sl, dl) for factor (sl*v + dl), or None
      const: overall constant if there are no factors at all (degenerate)
    Product of all factors (times const if no factors) equals H(v).
    """
    M = QSP_DEPTH + 1  # 28 nodes for degree 27
    vn = np.cos(np.pi * (np.arange(M) + 0.5) / M)
    h = _cheb.chebfit(vn, _qsp_f64(np.arccos(vn) / 2.0, phis), QSP_DEPTH)

    # effective degree (trim numerically-zero leading coefficients)
    tol = 1e-12 * max(np.abs(h).max(), 1e-30)
    deg = len(h) - 1
    while deg > 0 and abs(h[deg]) < tol:
        deg -= 1
    h = h[: deg + 1]
    if deg == 0:
        return [], None, float(h[0])

    r = _cheb.chebroots(h)
    lead = float(h[-1]) * 2.0 ** max(deg - 1, 0)  # monomial leading coef
    cplx = [z for z in r if abs(z.imag) > 1e-9]
    cplx = sorted((z for z in cplx if z.imag > 0), key=lambda z: z.real)
    real = sorted(z.real for z in r if abs(z.imag) <= 1e-9)

    raw = [(z.real, z.imag**2) for z in cplx]  # (p, q): (v-p)^2 + q
    lin_root = None
    if len(real) % 2 == 1:
        mid = len(real) // 2
        lin_root = real[mid]
        real = real[:mid] + real[mid + 1 :]
    for a, b2 in zip(real[0::2], real[1::2]):
        p = 0.5 * (a + b2)
        raw.append((p, a * b2 - p * p))

    # interleave small-|p| and large-|p| factors to keep partials balanced
    raw.sort(key=lambda pq: abs(pq[0]))
    order = []
    lo, hi = 0, len(raw) - 1
    while lo <= hi:
        order.append(raw[lo])
        if hi != lo:
            order.append(raw[hi])
        lo += 1
        hi -= 1

    grid = np.linspace(-1.0, 1.0, 4097)
    part = np.ones_like(grid)
    quads = []
    scale_left = lead  # product of remaining (unscaled) factor scales
    for p, q in order:
        f = (grid - p) ** 2 + q
        m = np.abs(part * f).max()
        a = 1.0 / m
        quads.append((float(np.sqrt(a)), float(-p * np.sqrt(a)), float(a * q)))
        part = part * f * a
        scale_left /= a

    if lin_root is not None:
        sl = scale_left
        return quads, (float(sl), float(-sl * lin_root)), None
    # even case: fold leftover scale into last quad
    sg, bg, cg = quads[-1]
    s = scale_left
    quads[-1] = (
        float(sg * np.sqrt(abs(s))),
        float(bg * np.sqrt(abs(s))),
        float(cg * abs(s)),
    )
    if s < 0:
        # negate one factor via c and flipping (sigma*v+beta)^2 sign is
        # impossible; instead fold sign into the first factor's c/sq by
        # negating the whole first factor: multiply y chain start by -1.
        sg0, bg0, cg0 = quads[0]
        quads[0] = (sg0, bg0, cg0)  # sign handled by caller via neg flag
    return quads, None, (-1.0 if s < 0 else 1.0)


def _simulate(theta, quads, lin, neg, alphas, bias, f16):
    """Bit-faithful host simulation of the device pipeline."""
    ft = np.float16 if f16 else np.float32
    th = theta.astype(np.float32)
    u = (th * np.float32(1.0 / _PI) + np.float32(0.25)).astype(np.float32)
    n = ((u + np.float32(_MAGIC)) - np.float32(_MAGIC)).astype(np.float32)
    f = (u - n).astype(np.float32)
    v = np.sin(2 * np.pi * f.astype(np.float64)).astype(np.float32)
    y = None
    for sg, bg, cg in quads:
        sq = ((np.float32(sg) * v + np.float32(bg)) ** 2).astype(ft)
        if y is None:
            y = (sq.astype(np.float32) + np.float32(cg)).astype(ft)
        else:
            y = ((sq.astype(np.float32) + np.float32(cg)) * y.astype(np.float32)).astype(ft)
    if lin is not None:
        lv = (np.float32(lin[0]) * v + np.float32(lin[1])).astype(ft)
        y = (y.astype(np.float32) * lv.astype(np.float32)).astype(ft)
    if neg is not None and neg < 0:
        y = (-y.astype(np.float32)).astype(ft)
    al = alphas.astype(ft if f16 else np.float32)
    out = (y.astype(np.float32) * al.astype(np.float32) + np.float32(bias)).astype(
        np.float32
    )
    return out


def _build_program(quads, lin, neg, bias_val, f16):
    """Build the per-core Bass program with all constants baked in."""
    nc = bass.Bass()
    cdt = dt.float16 if f16 else dt.float32

    x_d = nc.declare_dram_parameter("x", [P, F], dt.float32, isOutput=False)
    a_d = nc.declare_dram_parameter("alphas", [P, F], dt.float32, isOutput=False)
    c_d = nc.declare_dram_parameter("consts", [P, 32], dt.float32, isOutput=False)
    o_d = nc.declare_dram_parameter("out", [P, F], dt.float32, isOutput=True)

    nq = len(quads)
    # consts layout: [2pi, (beta_i, sigma_i)*nq, dl, sl]
    cvals = [2.0 * _PI]
    for sg, bg, cg in quads:
        cvals += [bg, sg]
    if lin is not None:
        cvals += [lin[1], lin[0]]
    consts = np.zeros(32, np.float32)
    consts[: len(cvals)] = np.array(cvals, np.float32)

    with (
        nc.sbuf_tensor([P, F], dt.float32) as th,
        nc.sbuf_tensor([P, F], dt.float32) as ut,
        nc.sbuf_tensor([P, F], dt.float32) as ntl,
        nc.sbuf_tensor([P, F], dt.float32) as ftl,
        nc.sbuf_tensor([P, F], dt.float32) as vt,
        nc.sbuf_tensor([P, 32], dt.float32) as ct,
        nc.sbuf_tensor([P, F * nq], cdt) as sqs,
        nc.sbuf_tensor([P, F], cdt) as lint,
        nc.sbuf_tensor([P, F], cdt) as ya,
        nc.sbuf_tensor([P, F], cdt) as yb,
        nc.sbuf_tensor([P, F], cdt) as al16,
        nc.sbuf_tensor([P, F], dt.float32) as ot,
        nc.semaphore() as dmax,
        nc.semaphore() as dmaa,
        nc.semaphore() as acts,
        nc.semaphore() as dves,
        nc.Block() as block,
    ):
        sq = [sqs[:, i * F : (i + 1) * F] for i in range(nq)]

        @block.sync
        def _(sync):
            sync.dma_start(out=ct[:], in_=c_d[:]).then_inc(dmax, 16)
            sync.dma_start(out=th[:], in_=x_d[:]).then_inc(dmax, 16)
            sync.wait_ge(dves, 2)
            sync.dma_start(out=o_d[:], in_=ot[:]).then_inc(dmax, 16)

        @block.gpsimd
        def _(gpsimd):
            gpsimd.dma_start(out=al16[:], in_=a_d[:]).then_inc(dmaa, 16)

        @block.scalar
        def _(scalar):
            scalar.wait_ge(dves, 1)
            scalar.wait_ge(dmax, 32)
            nc.scalar.activation(
                out=vt[:], in_=ftl[:], func=AF.Sin, scale=ct[:, 0:1]
            ).then_inc(acts, 1)
            for i in range(nq):
                nc.scalar.activation(
                    out=sq[i],
                    in_=vt[:],
                    func=AF.Square,
                    bias=ct[:, 1 + 2 * i : 2 + 2 * i],
                    scale=ct[:, 2 + 2 * i : 3 + 2 * i],
                ).then_inc(acts, 1)


        @block.vector
        def _(vector):
            vector.wait_ge(dmax, 32)
            nc.vector.tensor_scalar(
                out=ut[:], in0=th[:], scalar1=float(1.0 / _PI), scalar2=0.25,
                op0=AL.mult, op1=AL.add,
            )
            nc.vector.tensor_scalar(
                out=ntl[:], in0=ut[:], scalar1=_MAGIC, scalar2=_MAGIC,
                op0=AL.add, op1=AL.subtract,
            )
            nc.vector.tensor_tensor(
                out=ftl[:], in0=ut[:], in1=ntl[:], op=AL.subtract
            ).then_inc(dves, 1)

            ys = [ya, yb]
            y = None
            for i in range(nq):
                vector.wait_ge(acts, 2 + i)
                dst = ys[i % 2]
                if y is None:
                    nc.vector.tensor_scalar(
                        out=dst[:], in0=sq[0], scalar1=float(quads[0][2]),
                        scalar2=None, op0=AL.add,
                    )
                else:
                    nc.vector.scalar_tensor_tensor(
                        out=dst[:], in0=sq[i], scalar=float(quads[i][2]),
                        in1=y[:], op0=AL.add, op1=AL.mult,
                    )
                y = dst
            if lin is not None:
                nc.vector.tensor_scalar(
                    out=lint[:], in0=vt[:], scalar1=float(lin[0]),
                    scalar2=float(lin[1]), op0=AL.mult, op1=AL.add,
                )
                dst = ys[nq % 2]
                nc.vector.tensor_tensor(out=dst[:], in0=y[:], in1=lint[:], op=AL.mult)
                y = dst
            vector.wait_ge(dmaa, 16)
            sgn = -1.0 if (neg is not None and neg < 0) else 1.0
            if sgn < 0:
                inst = nc.vector.scalar_tensor_tensor(
                    out=ot[:], in0=al16[:], scalar=sgn, in1=y[:],
                    op0=AL.mult, op1=AL.mult,
                )
            else:
                inst = nc.vector.tensor_tensor(
                    out=ot[:], in0=y[:], in1=al16[:], op=AL.mult
                )
            if bias_val != 0.0:
                inst = nc.vector.tensor_scalar(
                    out=ot[:], in0=ot[:], scalar1=float(bias_val),
                    scalar2=None, op0=AL.add,
                )
            inst.then_inc(dves, 1)

    return nc, consts


def _run(x, qsp_params, alphas, bias, trace=False):
    theta = np.ascontiguousarray(x[:, 0], dtype=np.float32)
    alphas = np.ascontiguousarray(alphas, dtype=np.float32)
    phis = qsp_params.astype(np.float64)
    bias_val = float(np.asarray(bias).reshape(-1)[0])

    quads, lin, neg = _build_factors(phis)
    if not quads:
        out = (float(neg) * alphas + bias_val).astype(np.float32)[:, None]
        return out, None

    # choose fp16 vs fp32 chain by simulating on a subsample
    idx = np.linspace(0, B - 1, 32768).astype(np.int64)
    ref = _qsp_f64(theta[idx].astype(np.float64), phis) * alphas[idx] + bias_val
    scale = np.sqrt(np.mean(ref**2)) + 1e-12
    f16 = True
    sim = _simulate(theta[idx], quads, lin, neg, alphas[idx], bias_val, True)
    err = np.sqrt(np.mean((sim - ref) ** 2)) / scale
    if not np.isfinite(err) or err > 6e-3:
        f16 = False

    nc, consts = _build_program(quads, lin, neg, bias_val, f16)
    cbc = np.broadcast_to(consts, (P, 32)).copy()
    xs = theta.reshape(N_CORES, P, F)
    als = alphas.reshape(N_CORES, P, F)
    in_maps = [
        {"x": xs[i], "alphas": als[i], "consts": cbc} for i in range(N_CORES)
    ]
    res = run_bass_kernel_spmd(nc, in_maps, list(range(N_CORES)), trace=trace)
    out = np.concatenate([r["out"].reshape(-1) for r in res.results])
    return out.astype(np.float32)[:, None], res


def kernel(x, qsp_params, alphas, bias):
    out, _ = _run(x, qsp_params, alphas, bias)
    return out


# revision 9
# speedup vs baseline: 1.1802x; 1.1802x over previous
"""QSP KAN forward on 8 Trainium2 NeuronCores (Bass, data-parallel).

Math: with 2d X-rotations (d=27 -> 54 W factors), <0|U|0> contains only
even harmonics of theta, so

    qsp(theta) = H(cos 2*theta),   H = degree-27 Chebyshev polynomial.

H's coefficients follow from the 55 phases by interpolating the 2x2
recurrence at 28 nodes (O(55^2) host preprocessing of the replicated
phase vector). The series is truncated to the lowest degree that keeps
the (input-measured) truncation error under 2.5e-3 rel-rms, then factored
into real quadratics + an optional linear term via Chebyshev root finding.
Each quadratic is written (sigma*v + beta)^2 + c so one ScalarE Square
activation evaluates it; per-factor scales are balanced on a grid so all
fp16 intermediates stay O(1). A host fp16 simulation of the exact device
arithmetic gates the fp16 path (falls back to fp32 tiles if needed).

Device per core (65536 elements = one [128, 512] tile):
  x DMA is split across the three DMA-capable engines (sync / scalar /
  gpsimd queues) for ~3x load bandwidth; same for the (f16) output.
  DVE   range-reduces 2*theta+pi/2 into [-pi,pi] (magic-number rint),
        then runs the fused (sq_i + c_i) * y factor chain.
  ACT   one (preloaded-table) Sin + nq Square ops.
  POOL  computes lin(v) * alphas during the factor pipeline.
No collectives; pure data parallel over the batch.
"""

import numpy as np
from numpy.polynomial import chebyshev as _cheb

import concourse.bass as bass
import concourse.mybir as mybir
from concourse.bass_utils import run_bass_kernel_spmd

QSP_DEPTH = 27
N_PHIS = 2 * QSP_DEPTH + 1  # 55
B = 524288
N_CORES = 8
P, F = 128, 512  # per-core tile; P*F == B/N_CORES

_PI = float(np.pi)
_MAGIC = float(1.5 * 2**23)  # fp32 round-to-nearest-int magic constant
dt = mybir.dt
AF = mybir.ActivationFunctionType
AL = mybir.AluOpType

# free-dim thirds for the 3-way DMA splits
_SPLITS = [(0, 172), (172, 344), (344, 512)]


def _qsp_f64(theta, phis):
    """Reference QSP expectation, float64 (first row of the 2x2 chain)."""
    c = np.cos(theta)
    s = np.sin(theta)
    r0r = np.ones_like(theta)
    r0i = np.zeros_like(theta)
    r1r = np.zeros_like(theta)
    r1i = np.zeros_like(theta)
    for phi in phis[1:]:
        cp, sp = np.cos(phi), np.sin(phi)
        ar = r0r * c - r1i * s
        ai = r0i * c + r1r * s
        br = r1r * c - r0i * s
        bi = r1i * c + r0r * s
        r0r = ar * cp - ai * sp
        r0i = ar * sp + ai * cp
        r1r = br * cp + bi * sp
        r1i = bi * cp - br * sp
    return r0r * np.cos(phis[0]) - r0i * np.sin(phis[0])


def _build_factors(phis, v_sample, w_sample):
    """Truncate + factor H. Returns (quads, lin, neg) with
    quads = [(sigma, beta, c)], factor_i(v) = (sigma*v+beta)^2 + c,
    lin = (sl, dl) or None, neg = sign flag (or scalar for degenerate H).
    Product of factors (times neg if no factors) equals H_trunc(v)."""
    M = QSP_DEPTH + 1
    vn = np.cos(np.pi * (np.arange(M) + 0.5) / M)
    h_full = _cheb.chebfit(vn, _qsp_f64(np.arccos(vn) / 2.0, phis), QSP_DEPTH)

    # adaptive truncation, measured on the actual (v, alphas) sample
    ref = _cheb.chebval(v_sample, h_full) * w_sample
    scale = np.sqrt(np.mean(ref**2)) + 1e-12
    deg = len(h_full) - 1
    for d in range(4, deg + 1):
        yt = _cheb.chebval(v_sample, h_full[: d + 1]) * w_sample
        if np.sqrt(np.mean((yt - ref) ** 2)) / scale < 2.0e-3:
            deg = d
            break
    h = h_full[: deg + 1]
    tol = 1e-12 * max(np.abs(h).max(), 1e-30)
    while deg > 0 and abs(h[deg]) < tol:
        deg -= 1
    h = h[: deg + 1]
    if deg == 0:
        return [], None, float(h[0])

    r = _cheb.chebroots(h)
    lead = float(h[-1]) * 2.0 ** max(deg - 1, 0)
    cplx = sorted((z for z in r if abs(z.imag) > 1e-9 and z.imag > 0),
                  key=lambda z: z.real)
    real = sorted(z.real for z in r if abs(z.imag) <= 1e-9)

    raw = [(z.real, z.imag**2) for z in cplx]
    lin_root = None
    if len(real) % 2 == 1:
        mid = len(real) // 2
        lin_root = real[mid]
        real = real[:mid] + real[mid + 1 :]
    for a, b2 in zip(real[0::2], real[1::2]):
        p = 0.5 * (a + b2)
        raw.append((p, a * b2 - p * p))

    raw.sort(key=lambda pq: abs(pq[0]))
    order = []
    lo, hi = 0, len(raw) - 1
    while lo <= hi:
        order.append(raw[lo])
        if hi != lo:
            order.append(raw[hi])
        lo += 1
        hi -= 1

    grid = np.linspace(-1.0, 1.0, 4097)
    part = np.ones_like(grid)
    quads = []
    scale_left = lead
    for p, q in order:
        f = (grid - p) ** 2 + q
        a = 1.0 / np.abs(part * f).max()
        quads.append((float(np.sqrt(a)), float(-p * np.sqrt(a)), float(a * q)))
        part = part * f * a
        scale_left /= a

    if lin_root is not None:
        sl = scale_left
        return quads, (float(sl), float(-sl * lin_root)), None
    sg, bg, cg = quads[-1]
    s = scale_left
    quads[-1] = (
        float(sg * np.sqrt(abs(s))),
        float(bg * np.sqrt(abs(s))),
        float(cg * abs(s)),
    )
    return quads, None, (-1.0 if s < 0 else 1.0)


def _simulate(theta, quads, lin, neg, alphas, bias, f16):
    """Bit-faithful host simulation of the device pipeline."""
    ft = np.float16 if f16 else np.float32
    th = theta.astype(np.float32)
    u = (th * np.float32(1.0 / _PI) + np.float32(0.25)).astype(np.float32)
    n = ((u + np.float32(_MAGIC)) - np.float32(_MAGIC)).astype(np.float32)
    f = (u - n).astype(np.float32)
    v = np.sin(2 * np.pi * f.astype(np.float64)).astype(np.float32)
    y = None
    for sg, bg, cg in quads:
        sq = ((np.float32(sg) * v + np.float32(bg)) ** 2).astype(ft)
        t32 = sq.astype(np.float32) + np.float32(cg)
        y = t32.astype(ft) if y is None else (t32 * y.astype(np.float32)).astype(ft)
    al = alphas.astype(ft)
    if lin is not None:
        lv = (np.float32(lin[0]) * v + np.float32(lin[1])).astype(ft)
        la = (lv.astype(np.float32) * al.astype(np.float32)).astype(ft)
    else:
        sgn = np.float32(-1.0 if (neg is not None and neg < 0) else 1.0)
        la = (al.astype(np.float32) * sgn).astype(ft)
    out = (y.astype(np.float32) * la.astype(np.float32)).astype(ft)
    if bias != 0.0:
        out = (out.astype(np.float32) + np.float32(bias)).astype(ft)
    return out.astype(np.float32)


def _build_program(quads, lin, neg, bias_val, f16):
    """Build the per-core Bass program; all factor constants baked in."""
    nc = bass.Bass()
    cdt = dt.float16 if f16 else dt.float32

    x_d = nc.declare_dram_parameter("x", [P, F], dt.float32, isOutput=False)
    a_d = nc.declare_dram_parameter("alphas", [P, F], dt.float32, isOutput=False)
    c_d = nc.declare_dram_parameter("consts", [P, 32], dt.float32, isOutput=False)
    o_d = nc.declare_dram_parameter("out", [P, F], cdt, isOutput=True)

    nq = len(quads)
    cvals = [2.0 * _PI]
    for sg, bg, cg in quads:
        cvals += [bg, sg]
    consts = np.zeros(32, np.float32)
    consts[: len(cvals)] = np.array(cvals, np.float32)

    from contextlib import ExitStack

    with ExitStack() as stack:
        e = stack.enter_context
        th = e(nc.sbuf_tensor([P, F], dt.float32))
        ut = e(nc.sbuf_tensor([P, F], dt.float32))
        ntl = e(nc.sbuf_tensor([P, F], dt.float32))
        ftl = e(nc.sbuf_tensor([P, F], dt.float32))
        vt = e(nc.sbuf_tensor([P, F], dt.float32))
        ct = e(nc.sbuf_tensor([P, 32], dt.float32))
        scr = e(nc.sbuf_tensor([P, 1], dt.float32))
        sqs = e(nc.sbuf_tensor([P, F * nq], cdt))
        lint = e(nc.sbuf_tensor([P, F], cdt))
        lat = e(nc.sbuf_tensor([P, F], cdt))
        ya = e(nc.sbuf_tensor([P, F], cdt))
        yb = e(nc.sbuf_tensor([P, F], cdt))
        al16 = e(nc.sbuf_tensor([P, F], cdt))
        ot = e(nc.sbuf_tensor([P, F], cdt))
        dx = e(nc.semaphore())
        dc = e(nc.semaphore())
        da = e(nc.semaphore())
        acts = e(nc.semaphore())
        dves = e(nc.semaphore())
        pools = e(nc.semaphore())
        dout = e(nc.semaphore())
        block = e(nc.Block())
        sq = [sqs[:, i * F : (i + 1) * F] for i in range(nq)]
        (s1a, s1b), (s2a, s2b), (s3a, s3b) = _SPLITS

        @block.sync
        def _(sync):
            sync.dma_start(out=ct[:], in_=c_d[:]).then_inc(dc, 16)
            sync.dma_start(out=th[:, s1a:s1b], in_=x_d[:, s1a:s1b]).then_inc(dx, 16)
            sync.wait_ge(dves, 2)
            sync.dma_start(out=o_d[:, s1a:s1b], in_=ot[:, s1a:s1b]).then_inc(dout, 16)
            sync.wait_ge(dout, 16)

        @block.scalar
        def _(scalar):
            scalar.dma_start(out=th[:, s2a:s2b], in_=x_d[:, s2a:s2b]).then_inc(dx, 16)
            # table preload: dummy Sin on a [P,1] scratch (input garbage ok)
            nc.scalar.activation(out=scr[:], in_=scr[:], func=AF.Sin)
            scalar.wait_ge(dves, 1)
            scalar.wait_ge(dc, 16)
            nc.scalar.activation(
                out=vt[:], in_=ftl[:], func=AF.Sin, scale=ct[:, 0:1]
            ).then_inc(acts, 1)
            for i in range(nq):
                nc.scalar.activation(
                    out=sq[i],
                    in_=vt[:],
                    func=AF.Square,
                    bias=ct[:, 1 + 2 * i : 2 + 2 * i],
                    scale=ct[:, 2 + 2 * i : 3 + 2 * i],
                ).then_inc(acts, 1)
            scalar.wait_ge(dves, 2)
            scalar.dma_start(out=o_d[:, s2a:s2b], in_=ot[:, s2a:s2b]).then_inc(dout, 16)
            scalar.wait_ge(dout, 32)

        @block.gpsimd
        def _(gpsimd):
            gpsimd.dma_start(out=th[:, s3a:s3b], in_=x_d[:, s3a:s3b]).then_inc(dx, 16)
            gpsimd.dma_start(out=al16[:], in_=a_d[:]).then_inc(da, 16)
            gpsimd.wait_ge(acts, 1)
            gpsimd.wait_ge(da, 16)
            if lin is not None:
                nc.gpsimd.tensor_scalar(
                    out=lint[:], in0=vt[:], scalar1=float(lin[0]),
                    scalar2=float(lin[1]), op0=AL.mult, op1=AL.add,
                )
                nc.gpsimd.tensor_tensor(
                    out=lat[:], in0=lint[:], in1=al16[:], op=AL.mult
                ).then_inc(pools, 1)
            else:
                sgn = -1.0 if (neg is not None and neg < 0) else 1.0
                nc.gpsimd.tensor_scalar(
                    out=lat[:], in0=al16[:], scalar1=float(sgn), scalar2=None,
                    op0=AL.mult,
                ).then_inc(pools, 1)
            gpsimd.wait_ge(dves, 2)
            gpsimd.dma_start(out=o_d[:, s3a:s3b], in_=ot[:, s3a:s3b]).then_inc(dout, 16)
            gpsimd.wait_ge(dout, 48)

        @block.vector
        def _(vector):
            vector.wait_ge(dx, 48)
            nc.vector.tensor_scalar(
                out=ut[:], in0=th[:], scalar1=float(1.0 / _PI), scalar2=0.25,
                op0=AL.mult, op1=AL.add,
            )
            nc.vector.tensor_scalar(
                out=ntl[:], in0=ut[:], scalar1=_MAGIC, scalar2=_MAGIC,
                op0=AL.add, op1=AL.subtract,
            )
            nc.vector.tensor_tensor(
                out=ftl[:], in0=ut[:], in1=ntl[:], op=AL.subtract
            ).then_inc(dves, 1)

            ys = [ya, yb]
            y = None
            for i in range(nq):
                vector.wait_ge(acts, 2 + i)
                dst = ys[i % 2]
                if y is None:
                    nc.vector.tensor_scalar(
                        out=dst[:], in0=sq[0], scalar1=float(quads[0][2]),
                        scalar2=None, op0=AL.add,
                    )
                else:
                    nc.vector.scalar_tensor_tensor(
                        out=dst[:], in0=sq[i], scalar=float(quads[i][2]),
                        in1=y[:], op0=AL.add, op1=AL.mult,
                    )
                y = dst
            vector.wait_ge(pools, 1)
            if bias_val != 0.0:
                nc.vector.tensor_tensor(out=ot[:], in0=y[:], in1=lat[:], op=AL.mult)
                nc.vector.tensor_scalar(
                    out=ot[:], in0=ot[:], scalar1=float(bias_val), scalar2=None,
                    op0=AL.add,
                ).then_inc(dves, 2)
            else:
                nc.vector.tensor_tensor(
                    out=ot[:], in0=y[:], in1=lat[:], op=AL.mult
                ).then_inc(dves, 2)

    return nc, consts


def _run(x, qsp_params, alphas, bias, trace=False):
    theta = np.ascontiguousarray(x[:, 0], dtype=np.float32)
    alphas = np.ascontiguousarray(alphas, dtype=np.float32)
    phis = qsp_params.astype(np.float64)
    bias_val = float(np.asarray(bias).reshape(-1)[0])

    idx = np.linspace(0, B - 1, 32768).astype(np.int64)
    th_s = theta[idx].astype(np.float64)
    quads, lin, neg = _build_factors(phis, np.cos(2 * th_s), alphas[idx])
    if not quads:
        out = (float(neg) * alphas + bias_val).astype(np.float32)[:, None]
        return out, None

    ref = _qsp_f64(th_s, phis) * alphas[idx] + bias_val
    scale = np.sqrt(np.mean(ref**2)) + 1e-12
    sim = _simulate(theta[idx], quads, lin, neg, alphas[idx], bias_val, True)
    err = np.sqrt(np.mean((sim - ref) ** 2)) / scale
    f16 = bool(np.isfinite(err) and err < 8e-3)

    nc, consts = _build_program(quads, lin, neg, bias_val, f16)
    cbc = np.broadcast_to(consts, (P, 32)).copy()
    xs = theta.reshape(N_CORES, P, F)
    als = alphas.reshape(N_CORES, P, F)
    in_maps = [{"x": xs[i], "alphas": als[i], "consts": cbc} for i in range(N_CORES)]
    res = run_bass_kernel_spmd(nc, in_maps, list(range(N_CORES)), trace=trace)
    out = np.concatenate(
        [r["out"].astype(np.float32).reshape(-1) for r in res.results]
    )
    return out[:, None], res


def kernel(x, qsp_params, alphas, bias):
    out, _ = _run(x, qsp_params, alphas, bias)
    return out
